# revision 22
# baseline (speedup 1.0000x reference)
"""MoE output combine kernel for Trainium2 (Bass/Tile), 8-core SPMD.

Problem: out[b,s,e] = sum_n routing_logits[b,s,n] * expert_outputs[b,n,s,e]
  B=8, S=4096, N=8, E=128, fp32.

Sharding: batch across the 8 NeuronCores (fully local combine, no
collectives). Each core reads its (4096,8) logits slice and (8,4096,128)
expert slice (~16 MiB) and writes a (4096,128) output (~2 MiB) —
memory-bound at ~19 MB per core (~55us at the ~345GB/s/core HBM rate).

Layout: partition p holds s-rows [32p, 32p+32) — s = 32p + j, j in
[0,32). Every transfer is contiguous per partition; expert slabs load as
(128, 4096) tiles in 1MB halves (first two slabs in 512KB quarters so
compute starts early); logits load as one (128, 256) tile W[p, j*8+n].

Compute: the weight for free position (j, e) is W[p, j*8+n], constant
along e — expressed directly as a step-0 broadcast AP
  W[:, n::8][:, :, None].broadcast_to([128, 32, 128])
so a plain fp32 tensor_tensor at FD=2048/4096 (1 elem/lane/cycle, no
per-partition-scalar FD=128 limit, measured 4.42us @FD=4096) does each
expert's multiply in 2 ops and each accumulate in 2 ops:
  - Vector: expert 0 multiplies straight into the accumulator; experts
    1-4 via (tmp = x (.) Wb; acc += tmp) per 2048-wide half; then the
    scalar-engine product merge tree and per-group final accumulates
  - Scalar (ACT): experts 5-7 multiplied into tmp tiles with activation
    Copy + per-partition scale (FD=128, 477ns/op — ACT has no tensor-
    tensor path, so broadcast APs don't help it)
  - GpSimd: unused — its 2-input ops contend with the DVE for SBUF
    ports and inflate concurrent vector ops ~7x
Loads are ordered by each engine's need time; stores go per j-group
right after that group's final merge.
"""

import numpy as np

B, S, N, E = 8, 4096, 8, 128
P = 128
JF = S // P          # 32 j-values per partition
H = S // 2           # half-slab free width (2048)
Q = S // 4           # quarter width (1024)

V_EXPERTS = (1, 2, 3, 4)   # mult+add on vector engine (expert 0 also on V)
S_EXPERTS = (5, 6, 7)      # multiplied on scalar engine

_nc_cache = None


def _build_nc():
    import concourse.bacc as bacc
    import concourse.mybir as mybir
    from concourse.tile import TileContext

    f32 = mybir.dt.float32
    mult = mybir.AluOpType.mult
    add = mybir.AluOpType.add
    Copy = mybir.ActivationFunctionType.Copy

    nc = bacc.Bacc("TRN2", target_bir_lowering=False)
    logits = nc.dram_tensor("routing_logits", [S, N], f32, kind="ExternalInput")
    expert = nc.dram_tensor("expert_outputs", [N, S, E], f32, kind="ExternalInput")
    out = nc.dram_tensor("out", [S, E], f32, kind="ExternalOutput")

    with TileContext(nc) as tc:
        with (
            tc.tile_pool(name="xp", bufs=5) as xp,
            tc.tile_pool(name="tvp", bufs=2) as tvp,
            tc.tile_pool(name="cp", bufs=1) as cp,
        ):
            xs = {}

            def alloc_x(n):
                x = xp.tile([P, S], f32, tag="x")
                xs[n] = x

            def load_piece(n, lo, hi):
                src = expert[n].rearrange("(p j) e -> p (j e)", p=P)
                nc.sync.dma_start(out=xs[n][:, lo:hi], in_=src[:, lo:hi])

            w = cp.tile([P, JF * N], f32, tag="w")
            acc = cp.tile([P, S], f32, tag="acc")
            t5 = cp.tile([P, S], f32, tag="t5")
            t6 = cp.tile([P, S], f32, tag="t6")
            t7 = cp.tile([P, S], f32, tag="t7")
            ts_ = {5: t5, 6: t6, 7: t7}

            def wb(n, lo, hi):
                # broadcast weight view over free range [lo, hi) of the
                # (j e) layout: (128, nj, E) with step-0 innermost
                j0, j1 = lo // E, hi // E
                return w[:, n + 8 * j0 : n + 8 * (j1 - 1) + 1 : 8][:, :, None].broadcast_to(
                    [P, j1 - j0, E]
                )

            def j3(t, lo, hi):
                return t[:, lo:hi].rearrange("p (j e) -> p j e", e=E)

            def v_mult(dst, n, lo, hi):
                nc.vector.tensor_tensor(
                    out=j3(dst, lo, hi), in0=j3(xs[n], lo, hi), in1=wb(n, lo, hi),
                    op=mult,
                )

            def v_add(dst, src, lo, hi):
                nc.vector.tensor_tensor(
                    out=dst[:, lo:hi], in0=dst[:, lo:hi], in1=src[:, lo:hi], op=add
                )

            def v_expert_half(n, h):
                lo, hi = h * H, (h + 1) * H
                tv = tvp.tile([P, H], f32, tag="tv")
                v_mult_tv(tv, n, lo, hi)
                nc.vector.tensor_tensor(
                    out=acc[:, lo:hi], in0=acc[:, lo:hi], in1=tv[:], op=add
                )

            def v_expert(n):
                v_expert_half(n, 0)
                v_expert_half(n, 1)

            def v_mult_tv(tv, n, lo, hi):
                nc.vector.tensor_tensor(
                    out=tv[:].rearrange("p (j e) -> p j e", e=E),
                    in0=j3(xs[n], lo, hi), in1=wb(n, lo, hi), op=mult,
                )

            def s_wave(n):
                t = ts_[n]
                for j in range(JF):
                    nc.scalar.activation(
                        t[:, j * E : (j + 1) * E],
                        xs[n][:, j * E : (j + 1) * E],
                        Copy,
                        scale=w[:, j * N + n : j * N + n + 1],
                    )

            # dummy activation emitted before any DMA so the ACT function
            # table load lands in the preamble instead of behind the DMA
            # issue queue (it blocked all scalar work until ~20us)
            dummy = cp.tile([P, 1], f32, tag="dummy")
            nc.scalar.activation(dummy[:], dummy[:], Copy, scale=1.0)

            # 512KB-quarter load schedule found by simulating delivery vs
            # per-engine consumption times (serial ~353GB/s stream, V pace
            # ~2.29us/half, S pace ~3.8us/quarter-wave)
            LOAD_SEQ = [
                (0, 0), (0, 1), (0, 2), (0, 3), (1, 0), (5, 0), (1, 1), (5, 1),
                (1, 2), (5, 2), (1, 3), (5, 3), (2, 0), (2, 1), (6, 0), (2, 2),
                (2, 3), (6, 1), (3, 0), (3, 1), (6, 2), (6, 3), (3, 2), (3, 3),
                (4, 0), (7, 0), (4, 1), (4, 2), (7, 1), (7, 2), (7, 3), (4, 3),
            ]
            nc.sync.dma_start(out=w[:], in_=logits.rearrange("(p j) n -> p (j n)", p=P))

            def load_slots(lo, hi):
                # emit LOAD_SEQ entries [lo, hi) (0-based slot indices)
                for n, q in LOAD_SEQ[lo:hi]:
                    if n not in xs:
                        alloc_x(n)
                    load_piece(n, q * Q, (q + 1) * Q)

            out_r = out.rearrange("(p j) e -> p (j e)", p=P)

            def fin_store(g):
                lo, hi = g * Q, (g + 1) * Q
                v_add(acc, t5, lo, hi)
                nc.sync.dma_start(out=out_r[:, lo:hi], in_=acc[:, lo:hi])

            def v_expert_q(n, q):
                lo, hi = q * Q, (q + 1) * Q
                tv = tvp.tile([P, H], f32, tag="tv")
                nc.vector.tensor_tensor(
                    out=tv[:, :Q].rearrange("p (j e) -> p j e", e=E),
                    in0=j3(xs[n], lo, hi), in1=wb(n, lo, hi), op=mult,
                )
                nc.vector.tensor_tensor(
                    out=acc[:, lo:hi], in0=acc[:, lo:hi], in1=tv[:, :Q], op=add
                )

            # emission interleaves loads/compute in data-flow order (Tile
            # dependencies follow program order); per-engine queue orders
            # match the schedule search
            load_slots(0, 12)                 # x0, x5, x1 pieces
            for q in range(4):
                v_mult(acc, 0, q * Q, (q + 1) * Q)
            s_wave(5)
            load_slots(12, 18)                # x2, x6q0, x6q1
            v_expert(1)
            load_slots(18, 24)                # x3, x6q2, x6q3 (x3 reuses x0 slot)
            v_expert(2)
            s_wave(6)
            load_slots(24, 32)                # x4, x7 (reuse x5/x1 slots)
            # e3 with m56 h0 between its h0 mult and add
            tv3 = tvp.tile([P, H], f32, tag="tv")
            v_mult_tv(tv3, 3, 0, H)
            v_add(t5, t6, 0, H)               # m56 h0
            nc.vector.tensor_tensor(out=acc[:, 0:H], in0=acc[:, 0:H], in1=tv3[:], op=add)
            v_expert_half(3, 1)
            v_add(t5, t6, H, S)               # m56 h1
            s_wave(7)
            v_expert_q(4, 0)
            v_add(t5, t7, 0, H)               # m567 h0
            fin_store(0)
            v_expert_q(4, 1)
            fin_store(1)
            v_expert_q(4, 2)
            v_add(t5, t7, 2 * Q, 3 * Q)       # m567 q2
            fin_store(2)
            v_expert_q(4, 3)
            v_add(t5, t7, 3 * Q, S)           # m567 q3
            fin_store(3)
    nc.compile()
    return nc


def _get_nc():
    global _nc_cache
    if _nc_cache is None:
        _nc_cache = _build_nc()
    return _nc_cache


def kernel(routing_logits, expert_outputs, _trace=False):
    from concourse.bass_utils import run_bass_kernel_spmd

    logits = np.asarray(routing_logits, dtype=np.float32)
    expert = np.asarray(expert_outputs, dtype=np.float32)
    assert logits.shape == (B, S, N), logits.shape
    assert expert.shape == (B, N, S, E), expert.shape

    nc = _get_nc()
    in_maps = [
        {
            "routing_logits": np.ascontiguousarray(logits[b]),
            "expert_outputs": np.ascontiguousarray(expert[b]),
        }
        for b in range(B)
    ]
    res = run_bass_kernel_spmd(nc, in_maps, core_ids=list(range(B)), trace=_trace)
    out = np.stack([np.asarray(res.results[b]["out"]) for b in range(B)], axis=0)
    if _trace:
        return out, res
    return out


# revision 23
# speedup vs baseline: 1.1720x; 1.1720x over previous
"""MoE output combine kernel for Trainium2 (Bass/Tile), 8-core SPMD.

Problem: out[b,s,e] = sum_n routing_logits[b,s,n] * expert_outputs[b,n,s,e]
  B=8, S=4096, N=8, E=128, fp32.

Sharding: batch across the 8 NeuronCores (fully local combine, no
collectives). Each core reads its (4096,8) logits slice and (8,4096,128)
expert slice (~16 MiB) and writes a (4096,128) output (~2 MiB) —
memory-bound at ~19 MB per core (~55us at the ~345GB/s/core HBM rate).

Layout: partition p holds s-rows [32p, 32p+32) — s = 32p + j, j in
[0,32). Every transfer is contiguous per partition; expert slabs load as
(128, 4096) tiles in 1MB halves (first two slabs in 512KB quarters so
compute starts early); logits load as one (128, 256) tile W[p, j*8+n].

Compute: the weight for free position (j, e) is W[p, j*8+n], constant
along e — expressed directly as a step-0 broadcast AP
  W[:, n::8][:, :, None].broadcast_to([128, 32, 128])
so a plain fp32 tensor_tensor at FD=2048/4096 (1 elem/lane/cycle, no
per-partition-scalar FD=128 limit, measured 4.42us @FD=4096) does each
expert's multiply in 2 ops and each accumulate in 2 ops:
  - Vector: expert 0 multiplies straight into the accumulator; experts
    1-4 via (tmp = x (.) Wb; acc += tmp) per 2048-wide half; then the
    scalar-engine product merge tree and per-group final accumulates
  - Scalar (ACT): experts 5-7 multiplied into tmp tiles with activation
    Copy + per-partition scale (FD=128, 477ns/op — ACT has no tensor-
    tensor path, so broadcast APs don't help it)
  - GpSimd: unused — its 2-input ops contend with the DVE for SBUF
    ports and inflate concurrent vector ops ~7x
Loads are ordered by each engine's need time; stores go per j-group
right after that group's final merge.
"""

import numpy as np

B, S, N, E = 8, 4096, 8, 128
P = 128
JF = S // P          # 32 j-values per partition
H = S // 2           # half-slab free width (2048)
Q = S // 4           # quarter width (1024)

V_EXPERTS = (1, 2, 3, 4)   # mult+add on vector engine (expert 0 also on V)
S_EXPERTS = (5, 6, 7)      # multiplied on scalar engine

_nc_cache = None


def _build_nc():
    import concourse.bacc as bacc
    import concourse.mybir as mybir
    from concourse.tile import TileContext

    f32 = mybir.dt.float32
    mult = mybir.AluOpType.mult
    add = mybir.AluOpType.add
    Copy = mybir.ActivationFunctionType.Copy

    nc = bacc.Bacc("TRN2", target_bir_lowering=False)
    logits = nc.dram_tensor("routing_logits", [S, N], f32, kind="ExternalInput")
    expert = nc.dram_tensor("expert_outputs", [N, S, E], f32, kind="ExternalInput")
    out = nc.dram_tensor("out", [S, E], f32, kind="ExternalOutput")

    with TileContext(nc) as tc:
        with (
            tc.tile_pool(name="xp", bufs=5) as xp,
            tc.tile_pool(name="tvp", bufs=2) as tvp,
            tc.tile_pool(name="cp", bufs=1) as cp,
        ):
            xs = {}

            def alloc_x(n):
                x = xp.tile([P, S], f32, tag="x")
                xs[n] = x

            def load_piece(n, lo, hi):
                src = expert[n].rearrange("(p j) e -> p (j e)", p=P)
                nc.sync.dma_start(out=xs[n][:, lo:hi], in_=src[:, lo:hi])

            w = cp.tile([P, JF * N], f32, tag="w")
            acc = cp.tile([P, S], f32, tag="acc")
            t5 = cp.tile([P, S], f32, tag="t5")
            t6 = cp.tile([P, S], f32, tag="t6")
            t7 = cp.tile([P, S], f32, tag="t7")
            ts_ = {5: t5, 6: t6, 7: t7}

            def wb(n, lo, hi):
                # broadcast weight view over free range [lo, hi) of the
                # (j e) layout: (128, nj, E) with step-0 innermost
                j0, j1 = lo // E, hi // E
                return w[:, n + 8 * j0 : n + 8 * (j1 - 1) + 1 : 8][:, :, None].broadcast_to(
                    [P, j1 - j0, E]
                )

            def j3(t, lo, hi):
                return t[:, lo:hi].rearrange("p (j e) -> p j e", e=E)

            def v_mult(dst, n, lo, hi):
                nc.vector.tensor_tensor(
                    out=j3(dst, lo, hi), in0=j3(xs[n], lo, hi), in1=wb(n, lo, hi),
                    op=mult,
                )

            def v_add(dst, src, lo, hi):
                nc.vector.tensor_tensor(
                    out=dst[:, lo:hi], in0=dst[:, lo:hi], in1=src[:, lo:hi], op=add
                )

            def v_expert_half(n, h):
                lo, hi = h * H, (h + 1) * H
                tv = tvp.tile([P, H], f32, tag="tv")
                v_mult_tv(tv, n, lo, hi)
                nc.vector.tensor_tensor(
                    out=acc[:, lo:hi], in0=acc[:, lo:hi], in1=tv[:], op=add
                )

            def v_expert(n):
                v_expert_half(n, 0)
                v_expert_half(n, 1)

            def v_mult_tv(tv, n, lo, hi):
                nc.vector.tensor_tensor(
                    out=tv[:].rearrange("p (j e) -> p j e", e=E),
                    in0=j3(xs[n], lo, hi), in1=wb(n, lo, hi), op=mult,
                )

            def s_wave(n):
                t = ts_[n]
                for j in range(JF):
                    nc.scalar.activation(
                        t[:, j * E : (j + 1) * E],
                        xs[n][:, j * E : (j + 1) * E],
                        Copy,
                        scale=w[:, j * N + n : j * N + n + 1],
                    )

            # dummy activation emitted before any DMA so the ACT function
            # table load lands in the preamble instead of behind the DMA
            # issue queue (it blocked all scalar work until ~20us)
            dummy = cp.tile([P, 1], f32, tag="dummy")
            nc.scalar.activation(dummy[:], dummy[:], Copy, scale=1.0)

            # 512KB-quarter load schedule found by simulating delivery vs
            # per-engine consumption times (serial ~353GB/s stream, V pace
            # ~2.29us/half, S pace ~3.8us/quarter-wave)
            LOAD_SEQ = [
                (0, 0), (0, 1), (5, 0), (0, 2), (0, 3), (1, 0), (1, 1), (5, 1),
                (1, 2), (5, 2), (1, 3), (5, 3), (2, 0), (2, 1), (6, 0), (2, 2),
                (2, 3), (6, 1), (3, 0), (3, 1), (6, 2), (6, 3), (3, 2), (3, 3),
                (4, 0), (7, 0), (4, 1), (4, 2), (7, 1), (7, 2), (7, 3), (4, 3),
            ]
            nc.sync.dma_start(out=w[:], in_=logits.rearrange("(p j) n -> p (j n)", p=P))

            def load_slots(lo, hi):
                # emit LOAD_SEQ entries [lo, hi) (0-based slot indices)
                for n, q in LOAD_SEQ[lo:hi]:
                    if n not in xs:
                        alloc_x(n)
                    load_piece(n, q * Q, (q + 1) * Q)

            out_r = out.rearrange("(p j) e -> p (j e)", p=P)

            def fin_store(g):
                lo, hi = g * Q, (g + 1) * Q
                v_add(acc, t5, lo, hi)
                nc.sync.dma_start(out=out_r[:, lo:hi], in_=acc[:, lo:hi])

            def v_expert_q(n, q):
                lo, hi = q * Q, (q + 1) * Q
                tv = tvp.tile([P, H], f32, tag="tv")
                nc.vector.tensor_tensor(
                    out=tv[:, :Q].rearrange("p (j e) -> p j e", e=E),
                    in0=j3(xs[n], lo, hi), in1=wb(n, lo, hi), op=mult,
                )
                nc.vector.tensor_tensor(
                    out=acc[:, lo:hi], in0=acc[:, lo:hi], in1=tv[:, :Q], op=add
                )

            # emission interleaves loads/compute in data-flow order (Tile
            # dependencies follow program order); per-engine queue orders
            # match the schedule search
            load_slots(0, 12)                 # x0, x5, x1 pieces
            for q in range(4):
                v_mult(acc, 0, q * Q, (q + 1) * Q)
            s_wave(5)
            load_slots(12, 18)                # x2, x6q0, x6q1
            v_expert(1)
            load_slots(18, 24)                # x3, x6q2, x6q3 (x3 reuses x0 slot)
            v_expert(2)
            s_wave(6)
            load_slots(24, 32)                # x4, x7 (reuse x5/x1 slots)
            # e3 with m56 h0 between its h0 mult and add
            tv3 = tvp.tile([P, H], f32, tag="tv")
            v_mult_tv(tv3, 3, 0, H)
            v_add(t5, t6, 0, H)               # m56 h0
            nc.vector.tensor_tensor(out=acc[:, 0:H], in0=acc[:, 0:H], in1=tv3[:], op=add)
            v_expert_half(3, 1)
            v_add(t5, t6, H, S)               # m56 h1
            s_wave(7)
            v_expert_q(4, 0)
            v_add(t5, t7, 0, H)               # m567 h0
            fin_store(0)
            v_expert_q(4, 1)
            fin_store(1)
            v_expert_q(4, 2)
            v_add(t5, t7, 2 * Q, 3 * Q)       # m567 q2
            fin_store(2)
            v_expert_q(4, 3)
            v_add(t5, t7, 3 * Q, S)           # m567 q3
            fin_store(3)
    nc.compile()
    return nc


def _get_nc():
    global _nc_cache
    if _nc_cache is None:
        _nc_cache = _build_nc()
    return _nc_cache


def kernel(routing_logits, expert_outputs, _trace=False):
    from concourse.bass_utils import run_bass_kernel_spmd

    logits = np.asarray(routing_logits, dtype=np.float32)
    expert = np.asarray(expert_outputs, dtype=np.float32)
    assert logits.shape == (B, S, N), logits.shape
    assert expert.shape == (B, N, S, E), expert.shape

    nc = _get_nc()
    in_maps = [
        {
            "routing_logits": np.ascontiguousarray(logits[b]),
            "expert_outputs": np.ascontiguousarray(expert[b]),
        }
        for b in range(B)
    ]
    res = run_bass_kernel_spmd(nc, in_maps, core_ids=list(range(B)), trace=_trace)
    out = np.stack([np.asarray(res.results[b]["out"]) for b in range(B)], axis=0)
    if _trace:
        return out, res
    return out
